# revision 1
# baseline (speedup 1.0000x reference)
"""Trainium2 Bass kernel for LoRA-attention (nn_Attention_lorad).

Computes, for x: [8, 1024, 768]:
    qkv = x @ qkv_w.T + qkv_b           (only k = qkv[..., C:2C] is used)
    q   = lora_linear(x, q_w, q_b, q_A, q_B)
    v   = lora_linear(x, v_w, v_b, v_A, v_B)
    out = softmax(q k^T / sqrt(d)) v    per head (12 heads, d=64)
    y   = out @ proj_w.T + proj_b

Sharding: pure data-parallel over batch B=8 -> one batch element per core.

Host-side exact algebraic folds:
  - LoRA:   w_eff = w + (B @ A) / r           (removes LoRA matmuls on device)
  - v bias: softmax rows sum to 1, so P @ (v + 1 vb^T) = P @ v + 1 vb^T;
            hence pb_eff = proj_b + proj_w @ v_b and v is projected bias-free.
  - k bias: S[n,m] += q_n . kb is constant over keys m, so it cancels in
            softmax -> k is projected bias-free too (exact).

Device schedule (per core; all matmul operands bf16, f32 PSUM accum):
  1. qT/kT [c(jt), N] projected transposed (stationary w-block, moving xT);
     q bias added on DVE during PSUM eviction.
  2. Per head h: S.T[m,n] = kT_h x qT_h per m-tile -> ACT exp (scale 1/8)
     -> eps[mt] [128m, 1024n] bf16 in SBUF.
  3. v projected in natural orientation into v_aug [n(mt), 12*(64+1)] with a
     ones column per head; chains interleaved into early head slots as PE
     filler (2 PSUM banks).
  4. PV reoriented for full partition use: out[n-tile, 65] accumulates
     lhsT=eps[mt][:, n-slice] (stationary) x v_aug[mt] head slice (moving)
     over 8 m-tiles; col 64 = softmax denominator. PSUM start=True zeroes a
     whole 2KB bank, so the four 65-col regions per bank are DVE-memset once
     and accumulated with start=False. Normalization is a per-partition
     scalar mul (DVE) with DVE reciprocals, writing bf16 ao2[n,(nt,pair-c)].
  5. ao2 -> aoT [c, n] via DMA xbar transpose (free wrt PE/DVE/ACT).
  6. yT = pwT x aoT + pb_eff, split into two half-contractions (pairs
     [0,FSPLIT) into y_half, then the rest + an identity-matmul fold of
     y_half into PSUM) so only a short PE burst and a plain eviction trail
     the last transpose.

DMA granularity follows the cost model: each DMA instruction pays ~625ns on
a serialized HWDGE device and its bytes serialize on one DMA_ENGINES device,
so weights load as one DMA per used jt-slice (jt-major SBUF layout) and x as
two n-halves, ordered by first use. A short warmup matmul chain keeps the PE
p-state ramp from penalizing the first real projections.

No max-subtraction in softmax: logits are ~N(0,1) here (|logit| < ~8),
exp is safely within fp32/bf16 range; mathematically identical.
"""

import os
import sys

for _p in ("/opt/trn_rl_repo", "/root/.axon_site/_ro/trn_rl_repo"):
    if os.path.isdir(_p) and _p not in sys.path:
        sys.path.insert(0, _p)

import numpy as np
import ml_dtypes

import concourse.bacc as bacc
import concourse.mybir as mybir
from concourse.bass_utils import run_bass_kernel_spmd
from concourse.tile import TileContext
from contextlib import ExitStack

F32 = mybir.dt.float32
BF16 = mybir.dt.bfloat16
AFT = mybir.ActivationFunctionType

P = 128           # SBUF partitions
C = 768           # model dim
N = 1024          # sequence length
H = 12            # heads
D = 64            # head dim
R = 16            # lora rank
CT = C // P       # 6 c-tiles (= head pairs)
NT = N // P       # 8 token tiles
NCH = 2           # 512-wide chunks of N
CHUNK = N // NCH  # 512
VW = 384          # v projection chain width (2 chains per mt)
SCALE = D ** -0.5
LAG = 6           # eps buffers per m-tile tag (heads in flight)
VPLAN = {0: 3, 1: 3, 2: 3, 3: 3, 4: 2, 5: 2}
FSPLIT = 4        # final proj: pairs [0,FSPLIT) in half1, rest in half2

_CACHE = {}


def build_nc(use_f32r=True):
    nc = bacc.Bacc("TRN2", target_bir_lowering=False, debug=False)

    xT = nc.dram_tensor("xT", [C, N], BF16, kind="ExternalInput").ap()
    qwT = nc.dram_tensor("qwT", [C, C], BF16, kind="ExternalInput").ap()
    kwT = nc.dram_tensor("kwT", [C, C], BF16, kind="ExternalInput").ap()
    vwT = nc.dram_tensor("vwT", [C, C], BF16, kind="ExternalInput").ap()
    pwT = nc.dram_tensor("pwT", [C, C], BF16, kind="ExternalInput").ap()
    qb = nc.dram_tensor("qb", [P, CT], F32, kind="ExternalInput").ap()
    ident = nc.dram_tensor("ident", [P, P], BF16,
                           kind="ExternalInput").ap()
    pb = nc.dram_tensor("pb", [P, CT], F32, kind="ExternalInput").ap()
    yT = nc.dram_tensor("yT", [C, N], BF16,
                    kind="ExternalOutput").ap()

    with TileContext(nc) as tc, ExitStack() as ctx:
        persist = ctx.enter_context(tc.tile_pool(name="persist", bufs=1))
        xpool = ctx.enter_context(tc.tile_pool(name="xpool", bufs=1))
        qkpool = ctx.enter_context(tc.tile_pool(name="qkpool", bufs=2))
        epool = ctx.enter_context(tc.tile_pool(name="epool", bufs=1))
        apool = ctx.enter_context(tc.tile_pool(name="apool", bufs=1))
        small = ctx.enter_context(tc.tile_pool(name="small", bufs=2))
        fout = ctx.enter_context(tc.tile_pool(name="fout", bufs=4))
        pps = ctx.enter_context(tc.tile_pool(name="pps", bufs=2, space="PSUM"))
        sps = ctx.enter_context(tc.tile_pool(name="sps", bufs=2, space="PSUM"))

        # ---- long-lived SBUF tensors ----
        # jt-major weight layouts: one contiguous 768-col block per jt-slice
        # DMA, so subtile deps gate each qk_proj(jt) on exactly its own DMA.
        qw_big = persist.tile([P, CT * C], BF16, tag="qw", name="qw")
        kw_big = persist.tile([P, CT * C], BF16, tag="kw", name="kw")
        pw_big = persist.tile([P, CT * C], BF16, tag="pw", name="pw")
        x_big = xpool.tile([P, CT * N], BF16, tag="x", name="x")
        v_aug = [persist.tile([P, H * (D + 1)], BF16, tag=f"vaug{m}",
                              name=f"vaug{m}") for m in range(NT)]
        qb_sb = persist.tile([P, CT], F32, tag="qb", name="qb")
        id_sb = persist.tile([P, P], BF16, tag="ident", name="ident")
        pb_sb = persist.tile([P, CT], F32, tag="pb", name="pb")
        aoT_sb = [apool.tile([P, N], BF16, tag=f"aoT{t}", name=f"aoT{t}")
                  for t in range(CT)]
        y_half = [apool.tile([P, N], BF16, tag=f"yh{i}", name=f"yh{i}")
                  for i in range(CT)]

        def x_t(ct):
            return x_big[:, ct * N:(ct + 1) * N]

        def qw_t(jt, ct):
            return qw_big[:, jt * C + ct * P:jt * C + (ct + 1) * P]

        def kw_t(jt, ct):
            return kw_big[:, jt * C + ct * P:jt * C + (ct + 1) * P]

        def pw_t(ct, jt):
            return pw_big[:, ct * C + jt * P:ct * C + (jt + 1) * P]

        # v weights live only through the v projection (jc-major halves)
        vstack = ExitStack()
        vpool = vstack.enter_context(tc.tile_pool(name="vpool", bufs=1))
        vw_big = vpool.tile([P, CT * C], BF16, tag="vw", name="vw")
        vps = vstack.enter_context(
            tc.tile_pool(name="vps", bufs=2, space="PSUM"))

        def vw_t(jc, ct):
            return vw_big[:, jc * CT * VW + ct * VW:jc * CT * VW +
                          (ct + 1) * VW]

        # ---- input DMAs, ordered by first use ----
        def dma_x(ch, c0=0, c1=CT):
            nc.sync.dma_start(
                out=x_big.rearrange("p (ct n) -> p ct n", n=N)
                [:, c0:c1, ch * CHUNK:(ch + 1) * CHUNK],
                in_=xT[c0 * P:c1 * P, ch * CHUNK:(ch + 1) * CHUNK].rearrange(
                    "(ct p) n -> p ct n", p=P))

        def dma_w(dst_big, src, jt):
            nc.sync.dma_start(
                out=dst_big[:, jt * C:(jt + 1) * C].rearrange(
                    "p (ct c) -> p ct c", c=P),
                in_=src[:, jt * P:(jt + 1) * P].rearrange(
                    "(ct p) c -> p ct c", p=P))

        def dma_vw(jc):
            nc.sync.dma_start(
                out=vw_big[:, jc * CT * VW:(jc + 1) * CT * VW].rearrange(
                    "p (ct c) -> p ct c", c=VW),
                in_=vwT[:, jc * VW:(jc + 1) * VW].rearrange(
                    "(ct p) c -> p ct c", p=P))

        dma_x(0, 0, 3)
        dma_w(qw_big, qwT, 0)
        nc.sync.dma_start(out=qb_sb[:], in_=qb[:, :])
        dma_x(0, 3, CT)
        dma_x(1)
        dma_w(kw_big, kwT, 0)
        dma_w(qw_big, qwT, 1)
        dma_w(kw_big, kwT, 1)
        dma_vw(0)
        dma_w(qw_big, qwT, 2)
        dma_w(kw_big, kwT, 2)
        dma_vw(1)
        dma_w(qw_big, qwT, 3)
        dma_w(kw_big, kwT, 3)
        dma_w(qw_big, qwT, 4)
        dma_w(kw_big, kwT, 4)
        dma_w(qw_big, qwT, 5)
        dma_w(kw_big, kwT, 5)

        # warmup: dependency-free matmuls bridge the input-DMA latency so
        # the PE p-state is at peak when the first projection starts
        wsrc = persist.tile([P, CHUNK], BF16, tag="wsrc", name="wsrc")
        nc.vector.memset(wsrc[:], 0.0)
        for i in range(11):
            wp = pps.tile([P, CHUNK], F32, tag="pps", name="warm")
            nc.tensor.matmul(wp[:], lhsT=wsrc[:, 0:P], rhs=wsrc[:],
                             start=True, stop=True)

        # ones columns of v_aug (softmax denominator trick)
        ones_stage = persist.tile([P, H], F32, tag="ones", name="ones")
        nc.vector.memset(ones_stage[:], 1.0)
        for m in range(NT):
            ones_view = v_aug[m].rearrange("p (h s) -> p h s", s=D + 1)
            nc.gpsimd.tensor_copy(ones_view[:, :, D:D + 1], ones_stage[:])

        def qk_chunk(jt, dst, w_t, ch, biased):
            csl = slice(ch * CHUNK, (ch + 1) * CHUNK)
            ps = pps.tile([P, CHUNK], F32, tag="pps", name="pps")
            for ct in range(CT):
                nc.tensor.matmul(
                    ps[:], lhsT=w_t(jt, ct), rhs=x_t(ct)[:, csl],
                    start=(ct == 0), stop=(ct == CT - 1))
            if biased:
                nc.vector.tensor_scalar_add(dst[:, csl], ps[:],
                                            qb_sb[:, jt:jt + 1])
            else:
                nc.vector.tensor_copy(dst[:, csl], ps[:])

        def qk_proj(jt):
            qd = qkpool.tile([P, N], BF16, tag="qT", name="qT")
            kd = qkpool.tile([P, N], BF16, tag="kT", name="kT")
            for ch in range(NCH):
                qk_chunk(jt, qd, qw_t, ch, True)
                qk_chunk(jt, kd, kw_t, ch, False)
            return qd, kd

        def v_chain(mt, jc):
            """One v-projection chain: x[mt] block x vw cols -> v_aug[mt]."""
            ps = vps.tile([P, VW], F32, tag="vps", name="vps")
            for ct in range(CT):
                nc.tensor.matmul(
                    ps[:], lhsT=x_t(ct)[:, mt * P:(mt + 1) * P],
                    rhs=vw_t(jc, ct),
                    start=(ct == 0), stop=(ct == CT - 1))
            dst = v_aug[mt].rearrange("p (h s) -> p h s", s=D + 1)
            hpc = VW // D
            nc.vector.tensor_copy(
                dst[:, jc * hpc:(jc + 1) * hpc, 0:D],
                ps[:].rearrange("p (h s) -> p h s", s=D))

        def head_qk(h, qT_t, kT_t, mts):
            """QK matmuls + exps for head h over the given m-tiles."""
            o = D * (h % 2)
            eps = []
            for mt in mts:
                sp = sps.tile([P, N], F32, tag="sps", name="sps")
                for ch in range(NCH):
                    nc.tensor.matmul(
                        sp[:, ch * CHUNK:(ch + 1) * CHUNK],
                        lhsT=kT_t[o:o + D, mt * P:(mt + 1) * P],
                        rhs=qT_t[o:o + D, ch * CHUNK:(ch + 1) * CHUNK],
                        start=True, stop=True)
                ep = epool.tile([P, N], BF16, tag=f"e{mt}", name=f"e{mt}",
                                bufs=LAG)
                nc.scalar.activation(out=ep[:], in_=sp[:], func=AFT.Exp,
                                     scale=SCALE)
                eps.append(ep)
            return eps

        def pv_alloc():
            # PSUM start=True zeroes a whole 2KB bank, so 4 regions per bank
            # cannot each start their own group: memset the bank once and
            # accumulate with start=False throughout.
            pv = [pvps.tile([P, CHUNK], F32, tag=f"pv{i}", name=f"pv{i}")
                  for i in range(2)]
            for t in pv:
                nc.vector.memset(t[:, 0:4 * (D + 1)], 0.0)
            return pv

        def pv_mm(h, pv, eps, mts, last):
            for i, mt in enumerate(mts):
                vsl = v_aug[mt][:, h * (D + 1):(h + 1) * (D + 1)]
                for nt in range(NT):
                    t = pv[nt // 4]
                    o = (nt % 4) * (D + 1)
                    nc.tensor.matmul(
                        t[:, o:o + D + 1],
                        lhsT=eps[i][:, nt * P:(nt + 1) * P],
                        rhs=vsl,
                        start=False, stop=(last and mt == mts[-1]),
                        skip_group_check=True)

        def head_pv(h, eps, ao2):
            """Reoriented PV + normalization for one head -> ao2 columns."""
            pv = pv_alloc()
            pv_mm(h, pv, eps, range(NT), True)
            pv_norm(h, pv, ao2)

        def pv_norm(h, pv, ao2):
            hoff = D * (h % 2)
            rec = small.tile([P, NT], F32, tag="rec", name="rec")
            recv = rec.rearrange("p (q s) -> p q s", s=1)
            for i in range(2):
                pvv = pv[i][:, 0:4 * (D + 1)].rearrange(
                    "p (q s) -> p q s", s=D + 1)
                nc.vector.reciprocal(recv[:, 4 * i:4 * i + 4, :],
                                     pvv[:, 0:4, D:D + 1])
            ao2v = ao2.rearrange("p (nt c) -> p nt c", c=P)
            for nt in range(NT):
                t = pv[nt // 4]
                tv = t[:, 0:4 * (D + 1)].rearrange("p (q s) -> p q s",
                                                   s=D + 1)
                nc.vector.tensor_scalar_mul(
                    ao2v[:, nt:nt + 1, hoff:hoff + D],
                    tv[:, nt % 4:nt % 4 + 1, 0:D],
                    rec[:, nt:nt + 1])

        def transpose_pair(pair, ao2):
            nc.sync.dma_start_transpose(
                out=aoT_sb[pair].rearrange("c (nt nn) -> c nt nn",
                                           nn=P)[:, :, :],
                in_=ao2[:])

        # ---- emission schedule ----
        # Work-queue driven: filler closures (next jt projection chunks,
        # v chains, ready PVs, final-proj chunks) are popped between QK
        # m-tile pairs so the PE never head-of-line blocks on the sps
        # double buffer while the ACT exp stream paces the attention.
        pvps = None
        pending = {}   # h -> eps tiles
        ao2_t = None
        workq = []

        pv_done = set()

        def run_pv(hh):
            nonlocal ao2_t
            pv_done.add(hh)
            if hh % 2 == 0:
                ao2_t = small.tile([P, N], BF16, tag="ao2", name="ao2",
                                   bufs=2)
            head_pv(hh, pending.pop(hh), ao2_t)
            if hh % 2 == 1:
                transpose_pair(hh // 2, ao2_t)

        def enq_qkp(jt):
            qd = qkpool.tile([P, N], BF16, tag="qT", name="qT")
            kd = qkpool.tile([P, N], BF16, tag="kT", name="kT")
            workq.append((1.28, "p", lambda: qk_chunk(jt, qd, qw_t, 0, True)))
            workq.append((1.28, "p", lambda: qk_chunk(jt, kd, kw_t, 0, False)))
            workq.append((1.28, "p", lambda: qk_chunk(jt, qd, qw_t, 1, True)))
            workq.append((1.28, "p", lambda: qk_chunk(jt, kd, kw_t, 1, False)))
            return qd, kd

        def pop_filler(budget_us):
            while workq and budget_us > 0:
                cost, _, fn = workq.pop(0)
                fn()
                budget_us -= cost

        def final_a(jt, ch):
            csl = slice(ch * CHUNK, (ch + 1) * CHUNK)
            ps = pps.tile([P, CHUNK], F32, tag="pps", name="fps")
            for i, ct in enumerate(range(FSPLIT)):
                nc.tensor.matmul(
                    ps[:], lhsT=pw_t(ct, jt), rhs=aoT_sb[ct][:, csl],
                    start=(i == 0), stop=(ct == FSPLIT - 1))
            # alternate eviction engines: ACT idles once its exps drain
            if (jt + ch) % 2 == 0:
                nc.vector.tensor_scalar_add(
                    y_half[jt][:, csl], ps[:], pb_sb[:, jt:jt + 1])
            else:
                nc.scalar.activation(out=y_half[jt][:, csl], in_=ps[:],
                                     func=AFT.Identity,
                                     bias=pb_sb[:, jt:jt + 1])

        def final_b(jt):
            # wide chunk through the (now idle) score PSUM + one DMA; the
            # PSUM eviction + y_half combine alternates between a direct DVE
            # add (1.19us) and an ACT Identity eviction feeding a 4x-mode
            # all-SBUF bf16 DVE add (1.04 + 0.33, off the DVE serial path)
            ps = sps.tile([P, N], F32, tag="sps", name="f2")
            fold = jt % 2 == 0   # ACT evicts these; ACT cannot add y_half
            for ch in range(NCH):
                csl = slice(ch * CHUNK, (ch + 1) * CHUNK)
                for i, ct in enumerate(range(FSPLIT, CT)):
                    nc.tensor.matmul(
                        ps[:, csl], lhsT=pw_t(ct, jt),
                        rhs=aoT_sb[ct][:, csl],
                        start=(i == 0), stop=(not fold and ch == NCH - 1),
                        skip_group_check=True)
                if fold:
                    # y_half folded in on the PE via identity accumulate so
                    # the eviction is a plain ACT Identity copy
                    nc.tensor.matmul(
                        ps[:, csl], lhsT=id_sb[:], rhs=y_half[jt][:, csl],
                        start=False, stop=(ch == NCH - 1),
                        skip_group_check=True)
            ob = fout.tile([P, N], BF16, tag="ob", name="ob", bufs=4)
            if not fold:
                nc.vector.tensor_add(ob[:], ps[:], y_half[jt][:])
            else:
                nc.scalar.activation(out=ob[:], in_=ps[:], func=AFT.Identity)
            eng = nc.sync if jt % 2 == 1 else nc.scalar
            eng.dma_start(out=yT[jt * P:(jt + 1) * P, :], in_=ob[:])

        # PV pull-forward: PV(k) at slot 6 + ceil(k/2) (FIFO, ring-safe for
        # LAG=6); PV(11) and the last final half drain after the loop.
        pv_slot = {6: [0], 7: [1, 2], 8: [3, 4], 9: [5, 6, 7], 10: [8, 9]}
        vq = [(mt, jc) for mt in range(NT) for jc in range(2)]
        vi = 0

        # head 0 with interleaved projection so the ACT exp stream starts
        # as early as the DMAs allow
        qd0 = qkpool.tile([P, N], BF16, tag="qT", name="qT")
        kd0 = qkpool.tile([P, N], BF16, tag="kT", name="kT")
        qk_chunk(0, qd0, qw_t, 0, True)
        qk_chunk(0, kd0, kw_t, 0, False)
        qk_chunk(0, qd0, qw_t, 1, True)
        pending[0] = head_qk(0, qd0, kd0, range(4))
        qk_chunk(0, kd0, kw_t, 1, False)
        pending[0] += head_qk(0, qd0, kd0, range(4, NT))
        cur = (qd0, kd0)
        qk_next = None

        for h in range(H):
            jt = h // 2
            if h % 2 == 0 and h > 0:
                cur = qk_next
            q_t, k_t = cur

            # enqueue this slot's fillers
            if h % 2 == 1 and jt + 1 < CT:
                qk_next = enq_qkp(jt + 1)
            if h == 2:
                nc.sync.dma_start(out=id_sb[:], in_=ident[:, :])
                # pw needed only by the output projection: late emission
                # keeps its transfer off the serialized DMA device while
                # the attention inputs stream in
                nc.scalar.dma_start(
                    out=pw_big.rearrange("p (ct c) -> p ct c", c=C),
                    in_=pwT.rearrange("(ct p) c -> p ct c", p=P))
                nc.scalar.dma_start(out=pb_sb[:], in_=pb[:, :])
            for _ in range(VPLAN.get(h, 0)):
                mt, jc = vq[vi]
                vi += 1
                workq.append((0.96, "v", lambda m=mt, j=jc: v_chain(m, j)))
            for k in pv_slot.get(h, []):
                workq.append((1.78, "pv", lambda kk=k: run_pv(kk)))
            # eps ring: head h's exps reuse head h-LAG's buffers; that PV
            # must be emitted first or the engines deadlock
            if h - LAG >= 0:
                while h - LAG not in pv_done:
                    cost, _, fn = workq.pop(0)
                    fn()

            if h == 10:
                for jj in range(2):
                    for cc in range(NCH):
                        workq.append((0.85, "f", lambda a=jj, b=cc: final_a(a, b)))

            if h == 11:
                # all queued PVs must precede PV(10)/PV(11) in the pvps
                # tag rotation or they deadlock behind norm(11)
                while any(k == "pv" for _, k, _ in workq):
                    cost, _, fn = workq.pop(0)
                    fn()
                # drain PV(10), then accumulate PV(11) incrementally behind
                # each pair of its own exps so only 2 m-tiles trail the
                # last exp
                run_pv(10)
                ao2_11 = ao2_t
                pv11 = pv_alloc()
                for mtp in range(4):
                    eps2 = head_qk(h, q_t, k_t, range(2 * mtp, 2 * mtp + 2))
                    if mtp == 3:
                        # fillers ahead of the exp-gated last PV pieces so
                        # the PE computes through the ACT exp trail
                        pop_filler(2.2)
                    pv_mm(h, pv11, eps2, range(2 * mtp, 2 * mtp + 2),
                          mtp == 3)
                    if mtp < 3:
                        pop_filler(1.0)
            elif h > 0:
                for mtp in range(4):
                    pending.setdefault(h, [])
                    pending[h] += head_qk(h, q_t, k_t,
                                          range(2 * mtp, 2 * mtp + 2))
                    pop_filler(1.0)
            if h == 5:
                # all v chains must be emitted before the v pools close
                while any(k == "v" for _, k, _ in workq):
                    cost, _, fn = workq.pop(0)
                    fn()
                vstack.close()
                pvps = ctx.enter_context(
                    tc.tile_pool(name="pvps", bufs=1, space="PSUM"))

        pv_norm(11, pv11, ao2_11)
        transpose_pair(5, ao2_11)
        # remaining fillers + deferred stage-A cover the transpose latency
        pop_filler(1e9)
        for jt in range(CT):
            if jt + 2 < CT:
                for cc in range(NCH):
                    final_a(jt + 2, cc)
            final_b(jt)

    nc.compile()
    return nc


def _get_nc(use_f32r=True):
    key = ("nc", use_f32r)
    if key not in _CACHE:
        _CACHE[key] = build_nc(use_f32r)
    return _CACHE[key]


def kernel(x, qkv_w, qkv_b, q_w, q_b, q_A, q_B, v_w, v_b, v_A, v_B,
           proj_w, proj_b, _trace=False, _use_f32r=True):
    x = np.ascontiguousarray(np.asarray(x, dtype=np.float32))
    B = x.shape[0]
    assert x.shape == (8, N, C)

    qkv_w = np.asarray(qkv_w, np.float32)
    q_w = np.asarray(q_w, np.float32)
    q_b = np.asarray(q_b, np.float32)
    q_A = np.asarray(q_A, np.float32)
    q_B = np.asarray(q_B, np.float32)
    v_w = np.asarray(v_w, np.float32)
    v_b = np.asarray(v_b, np.float32)
    v_A = np.asarray(v_A, np.float32)
    v_B = np.asarray(v_B, np.float32)
    proj_w = np.asarray(proj_w, np.float32)
    proj_b = np.asarray(proj_b, np.float32)

    # exact algebraic folds (see module docstring)
    qw_eff = q_w + (q_B @ q_A) * (1.0 / R)
    vw_eff = v_w + (v_B @ v_A) * (1.0 / R)
    kw = qkv_w[C:2 * C]
    pb_eff = proj_b + proj_w @ v_b

    bf = ml_dtypes.bfloat16
    common = {
        "qwT": np.ascontiguousarray(qw_eff.T.astype(bf)),
        "kwT": np.ascontiguousarray(kw.T.astype(bf)),
        "vwT": np.ascontiguousarray(vw_eff.T.astype(bf)),
        "pwT": np.ascontiguousarray(proj_w.T.astype(bf)),
        "qb": np.ascontiguousarray(q_b.reshape(CT, P).T),
        "ident": np.eye(P, dtype=bf),
        "pb": np.ascontiguousarray(pb_eff.reshape(CT, P).T),
    }
    in_maps = [
        {"xT": np.ascontiguousarray(x[i].T.astype(bf)), **common}
        for i in range(B)
    ]

    nc = _get_nc(_use_f32r)
    res = run_bass_kernel_spmd(nc, in_maps, list(range(B)), trace=_trace)

    out = np.empty((B, N, C), np.float32)
    for i in range(B):
        out[i] = np.asarray(res.results[i]["yT"], np.float32).T
    if _trace:
        return out, res
    return out



# revision 7
# speedup vs baseline: 1.0320x; 1.0320x over previous
"""Trainium2 Bass kernel for LoRA-attention (nn_Attention_lorad).

Computes, for x: [8, 1024, 768]:
    qkv = x @ qkv_w.T + qkv_b           (only k = qkv[..., C:2C] is used)
    q   = lora_linear(x, q_w, q_b, q_A, q_B)
    v   = lora_linear(x, v_w, v_b, v_A, v_B)
    out = softmax(q k^T / sqrt(d)) v    per head (12 heads, d=64)
    y   = out @ proj_w.T + proj_b

Sharding: pure data-parallel over batch B=8 -> one batch element per core.

Host-side exact algebraic folds (as the bf16 baseline):
  - LoRA:   w_eff = w + (B @ A) / r
  - v bias: pb_eff = proj_b + proj_w @ v_b  (softmax rows sum to 1)
  - k bias: constant over keys -> cancels in softmax (exact)

Structure follows the tuned bf16 baseline (same pools, work-queue pacing,
eps ring, PV orientation, final-projection split); two changes:

1. q/k/v projections run as SPLIT-fp8 DoubleRow matmuls: x and W are
   decomposed host-side into fp8e4m3 hi + lo parts (x = xh + xl exactly to
   fp8^2 precision); the device computes wh*xh + wh*xl + wl*xh as three
   DoubleRow passes, each contracting 2x128 rows per step at 0.5 PE
   cycles/column -> 0.75x the bf16 cycle count with BETTER-than-bf16
   accuracy (dropped xl*wl term ~0.04%). QK/PV/final stay bf16: their
   contractions (64 / 128 wide) cannot pack DoubleRow slots without
   either accuracy loss (single fp8 noise passes 1:1 into the output:
   attention output of a diffuse softmax is itself O(sqrt(sum p^2)), so
   relative weight noise does NOT average away) or losing the 2x again
   on extra hi/lo terms.

2. A small share of softmax exps runs on DVE instead of ACT, as a
   Schraudolph-style exp: i16 = S*c1 + c2 written as int16 and bitcast
   to bf16 (the integer IS the bf16 exponent+mantissa; ~1.8% rms sawtooth
   error on those tiles only). This decouples the PE's S-tile PSUM
   rotation from ACT's bursty exp queue. The share is kept small (~1/7)
   to bound the added output error (~0.7%).

DMA granularity follows the cost model: weights jt-sliced with hi/lo
adjacent so each projection gates on exactly its own DMA; non-critical
DMAs (pw, ident, pb, y-out) ride the otherwise-idle Pool queue whose
DGE setup is cheap.
"""

import os
import sys

for _p in ("/opt/trn_rl_repo", "/root/.axon_site/_ro/trn_rl_repo"):
    if os.path.isdir(_p) and _p not in sys.path:
        sys.path.insert(0, _p)

import numpy as np
import ml_dtypes

import concourse.bacc as bacc
import concourse.mybir as mybir
from concourse.bass_utils import run_bass_kernel_spmd
from concourse.tile import TileContext
from contextlib import ExitStack

F32 = mybir.dt.float32
BF16 = mybir.dt.bfloat16
I16 = mybir.dt.int16
FP8 = mybir.dt.float8e4
AFT = mybir.ActivationFunctionType
ALU = mybir.AluOpType
DR = mybir.MatmulPerfMode.DoubleRow

P = 128           # SBUF partitions
C = 768           # model dim
N = 1024          # sequence length
H = 12            # heads
D = 64            # head dim
R = 16            # lora rank
CT = C // P       # 6 c-tiles (= head pairs)
NT = N // P       # 8 token tiles
NCH = 2           # 512-wide chunks of N
CHUNK = N // NCH  # 512
VW = 384          # v projection chain width (2 chains per mt)
SCALE = D ** -0.5
LAG = 6           # eps buffers per m-tile tag (heads in flight)
VPLAN = {0: 3, 1: 3, 2: 3, 3: 3, 4: 2, 5: 2}
FSPLIT = 4        # final proj: pairs [0,FSPLIT) in half1, rest in half2

SX = 32.0         # x fp8 scale
SW = 256.0        # weight fp8 scale
EVQ = float(1.0 / (SX * SW))                # projection eviction scale
SC1 = float(np.log2(np.e) * 128.0 * SCALE)  # schraudolph mult (bf16 exp)
SC2 = float(127.0 * 128.0 - 7.25)           # schraudolph bias (rms-opt)

# which (head, mt) exps run on DVE via schraudolph; rest on ACT natively
def _exp_on_dve(h, mt):
    return (h * NT + mt) % 7 == 3

_CACHE = {}


def build_nc(use_f32r=True):
    nc = bacc.Bacc("TRN2", target_bir_lowering=False, debug=False)

    # hi/lo fp8 packs: x8 [p, hl, s, i, n]; wq8/wk8 [p, jt, hl, s, i, 128];
    # wv8 [p, hl, s, i, 768]
    x8d = nc.dram_tensor("x8", [P, 2 * 6 * N], FP8, kind="ExternalInput").ap()
    wq8d = nc.dram_tensor("wq8", [P, 6 * 2 * 6 * P], FP8,
                          kind="ExternalInput").ap()
    wk8d = nc.dram_tensor("wk8", [P, 6 * 2 * 6 * P], FP8,
                          kind="ExternalInput").ap()
    wv8d = nc.dram_tensor("wv8", [P, 2 * 6 * C], FP8,
                          kind="ExternalInput").ap()
    pwT = nc.dram_tensor("pwT", [C, C], BF16, kind="ExternalInput").ap()
    qb = nc.dram_tensor("qb", [P, CT], F32, kind="ExternalInput").ap()
    ident = nc.dram_tensor("ident", [P, P], BF16,
                           kind="ExternalInput").ap()
    pb = nc.dram_tensor("pb", [P, CT], F32, kind="ExternalInput").ap()
    yT = nc.dram_tensor("yT", [C, N], BF16,
                    kind="ExternalOutput").ap()

    with TileContext(nc) as tc, ExitStack() as ctx:
        persist = ctx.enter_context(tc.tile_pool(name="persist", bufs=1))
        xpool = ctx.enter_context(tc.tile_pool(name="xpool", bufs=1))
        qkpool = ctx.enter_context(tc.tile_pool(name="qkpool", bufs=2))
        epool = ctx.enter_context(tc.tile_pool(name="epool", bufs=1))
        apool = ctx.enter_context(tc.tile_pool(name="apool", bufs=1))
        small = ctx.enter_context(tc.tile_pool(name="small", bufs=2))
        fout = ctx.enter_context(tc.tile_pool(name="fout", bufs=4))
        pps = ctx.enter_context(tc.tile_pool(name="pps", bufs=2, space="PSUM"))
        sps = ctx.enter_context(tc.tile_pool(name="sps", bufs=2, space="PSUM"))

        # ---- long-lived SBUF tensors ----
        qw_big = persist.tile([P, 6 * 2 * 6 * P], FP8, tag="qw", name="qw")
        kw_big = persist.tile([P, 6 * 2 * 6 * P], FP8, tag="kw", name="kw")
        pw_big = persist.tile([P, CT * C], BF16, tag="pw", name="pw")
        x_big = xpool.tile([P, 2 * 6 * N], FP8, tag="x", name="x")
        v_aug = [persist.tile([P, H * (D + 1)], BF16, tag=f"vaug{m}",
                              name=f"vaug{m}") for m in range(NT)]
        qb_sb = persist.tile([P, CT], F32, tag="qb", name="qb")
        id_sb = persist.tile([P, P], BF16, tag="ident", name="ident")
        pb_sb = persist.tile([P, CT], F32, tag="pb", name="pb")
        aoT_sb = [apool.tile([P, N], BF16, tag=f"aoT{t}", name=f"aoT{t}")
                  for t in range(CT)]
        y_half = [apool.tile([P, N], BF16, tag=f"yh{i}", name=f"yh{i}")
                  for i in range(CT)]

        xv = x_big.rearrange("p (hl s i n) -> p hl s i n", hl=2, s=3, n=N)
        qwv = qw_big.rearrange("p (jt hl s i q) -> p jt hl s i q",
                               jt=6, hl=2, s=3, q=P)
        kwv = kw_big.rearrange("p (jt hl s i q) -> p jt hl s i q",
                               jt=6, hl=2, s=3, q=P)

        def pw_t(ct, jt):
            return pw_big[:, ct * C + jt * P:ct * C + (jt + 1) * P]

        # v weights live only through the v projection
        vstack = ExitStack()
        vpool = vstack.enter_context(tc.tile_pool(name="vpool", bufs=1))
        vw_big = vpool.tile([P, 2 * 6 * C], FP8, tag="vw", name="vw")
        vwv = vw_big.rearrange("p (hl s i j) -> p hl s i j", hl=2, s=3, j=C)
        vps = vstack.enter_context(
            tc.tile_pool(name="vps", bufs=2, space="PSUM"))

        # ---- input DMAs, ordered by first use ----
        x8dv = x8d.rearrange("p (hl s i n) -> p hl s i n", hl=2, s=3, n=N)

        def dma_x(ch):
            nsl = slice(ch * CHUNK, (ch + 1) * CHUNK)
            nc.sync.dma_start(out=xv[:, :, :, :, nsl],
                              in_=x8dv[:, :, :, :, nsl])

        def dma_w(dst, src, jt):
            sl = slice(jt * 2 * 6 * P, (jt + 1) * 2 * 6 * P)
            nc.sync.dma_start(out=dst[:, sl], in_=src[:, sl])

        dma_x(0)
        dma_w(qw_big, wq8d, 0)
        nc.sync.dma_start(out=qb_sb[:], in_=qb[:, :])
        dma_x(1)
        dma_w(kw_big, wk8d, 0)
        dma_w(qw_big, wq8d, 1)
        dma_w(kw_big, wk8d, 1)
        nc.sync.dma_start(out=vw_big[:, 0:6 * C], in_=wv8d[:, 0:6 * C])
        dma_w(qw_big, wq8d, 2)
        dma_w(kw_big, wk8d, 2)
        nc.sync.dma_start(out=vw_big[:, 6 * C:], in_=wv8d[:, 6 * C:])
        dma_w(qw_big, wq8d, 3)
        dma_w(kw_big, wk8d, 3)
        dma_w(qw_big, wq8d, 4)
        dma_w(kw_big, wk8d, 4)
        dma_w(qw_big, wq8d, 5)
        dma_w(kw_big, wk8d, 5)

        # warmup: dependency-free matmuls bridge the input-DMA latency so
        # real projections start past the sim's 3us mid-p-state window
        wsrc = persist.tile([P, CHUNK], BF16, tag="wsrc", name="wsrc")
        nc.vector.memset(wsrc[:], 0.0)
        for i in range(11):
            wp = pps.tile([P, CHUNK], F32, tag="pps", name="warm")
            nc.tensor.matmul(wp[:], lhsT=wsrc[:, 0:P], rhs=wsrc[:],
                             start=True, stop=True)

        # ones columns of v_aug (softmax denominator trick)
        ones_stage = persist.tile([P, H], F32, tag="ones", name="ones")
        nc.vector.memset(ones_stage[:], 1.0)
        for m in range(NT):
            ones_view = v_aug[m].rearrange("p (h s) -> p h s", s=D + 1)
            nc.gpsimd.tensor_copy(ones_view[:, :, D:D + 1], ones_stage[:])

        # split-fp8 term order: (w_hl, x_hl) = hh, h(w)l(x), l(w)h(x)
        TERMS = ((0, 0), (0, 1), (1, 0))

        def qk_chunk(jt, dst, w_v, ch, biased):
            """512-col chunk of a q/k projection via 3-term split-fp8 DR."""
            csl = slice(ch * CHUNK, (ch + 1) * CHUNK)
            ps = pps.tile([P, CHUNK], F32, tag="pps", name="pps")
            for s in range(3):
                for ti, (wl, xl) in enumerate(TERMS):
                    for sub in range(2):
                        nc.tensor.matmul(
                            ps[:, sub * 256:(sub + 1) * 256],
                            lhsT=w_v[:, jt, wl, s],
                            rhs=xv[:, xl, s, :,
                                   ch * CHUNK + sub * 256:
                                   ch * CHUNK + (sub + 1) * 256],
                            start=(s == 0 and ti == 0 and sub == 0),
                            stop=(s == 2 and ti == 2 and sub == 1),
                            perf_mode=DR, skip_group_check=True)
            if biased:
                nc.vector.tensor_scalar(
                    out=dst[:, csl], in0=ps[:], scalar1=EVQ,
                    scalar2=qb_sb[:, jt:jt + 1],
                    op0=ALU.mult, op1=ALU.add)
            else:
                nc.vector.tensor_scalar_mul(dst[:, csl], ps[:], EVQ)

        def qk_proj(jt):
            qd = qkpool.tile([P, N], BF16, tag="qT", name="qT")
            kd = qkpool.tile([P, N], BF16, tag="kT", name="kT")
            for ch in range(NCH):
                qk_chunk(jt, qd, qwv, ch, True)
                qk_chunk(jt, kd, kwv, ch, False)
            return qd, kd

        def v_chain(mt, jc):
            """One v-projection chain: x[mt] block x vw col-half (DR)."""
            ps = vps.tile([P, VW], F32, tag="vps", name="vps")
            for s in range(3):
                for ti, (wl, xl) in enumerate(TERMS):
                    for sub in range(2):
                        nc.tensor.matmul(
                            ps[:, sub * 192:(sub + 1) * 192],
                            lhsT=xv[:, xl, s, :, mt * P:(mt + 1) * P],
                            rhs=vwv[:, wl, s, :,
                                    jc * VW + sub * 192:
                                    jc * VW + (sub + 1) * 192],
                            start=(s == 0 and ti == 0 and sub == 0),
                            stop=(s == 2 and ti == 2 and sub == 1),
                            perf_mode=DR, skip_group_check=True)
            dst = v_aug[mt].rearrange("p (h s) -> p h s", s=D + 1)
            hpc = VW // D
            nc.vector.tensor_scalar_mul(
                dst[:, jc * hpc:(jc + 1) * hpc, 0:D],
                ps[:].rearrange("p (h s) -> p h s", s=D), EVQ)

        def head_qk(h, qT_t, kT_t, mts):
            """QK matmuls + exps for head h over the given m-tiles."""
            o = D * (h % 2)
            eps = []
            for mt in mts:
                sp = sps.tile([P, N], F32, tag="sps", name="sps")
                for ch in range(NCH):
                    nc.tensor.matmul(
                        sp[:, ch * CHUNK:(ch + 1) * CHUNK],
                        lhsT=kT_t[o:o + D, mt * P:(mt + 1) * P],
                        rhs=qT_t[o:o + D, ch * CHUNK:(ch + 1) * CHUNK],
                        start=True, stop=True)
                ep = epool.tile([P, N], BF16, tag=f"e{mt}", name=f"e{mt}",
                                bufs=LAG)
                if _exp_on_dve(h, mt):
                    nc.vector.tensor_scalar(
                        out=ep.bitcast(I16)[:], in0=sp[:],
                        scalar1=SC1, scalar2=SC2,
                        op0=ALU.mult, op1=ALU.add)
                else:
                    nc.scalar.activation(out=ep[:], in_=sp[:], func=AFT.Exp,
                                         scale=SCALE)
                eps.append(ep)
            return eps

        def pv_alloc():
            # PSUM start=True zeroes a whole 2KB bank, so 4 regions per bank
            # cannot each start their own group: memset the bank once and
            # accumulate with start=False throughout.
            pv = [pvps.tile([P, CHUNK], F32, tag=f"pv{i}", name=f"pv{i}")
                  for i in range(2)]
            for t in pv:
                nc.vector.memset(t[:, 0:4 * (D + 1)], 0.0)
            return pv

        def pv_mm(h, pv, eps, mts, last):
            for i, mt in enumerate(mts):
                vsl = v_aug[mt][:, h * (D + 1):(h + 1) * (D + 1)]
                for nt in range(NT):
                    t = pv[nt // 4]
                    o = (nt % 4) * (D + 1)
                    nc.tensor.matmul(
                        t[:, o:o + D + 1],
                        lhsT=eps[i][:, nt * P:(nt + 1) * P],
                        rhs=vsl,
                        start=False, stop=(last and mt == mts[-1]),
                        skip_group_check=True)

        def head_pv(h, eps, ao2):
            """Reoriented PV + normalization for one head -> ao2 columns."""
            pv = pv_alloc()
            pv_mm(h, pv, eps, range(NT), True)
            pv_norm(h, pv, ao2)

        def pv_norm(h, pv, ao2):
            hoff = D * (h % 2)
            rec = small.tile([P, NT], F32, tag="rec", name="rec")
            recv = rec.rearrange("p (q s) -> p q s", s=1)
            for i in range(2):
                pvv = pv[i][:, 0:4 * (D + 1)].rearrange(
                    "p (q s) -> p q s", s=D + 1)
                nc.vector.reciprocal(recv[:, 4 * i:4 * i + 4, :],
                                     pvv[:, 0:4, D:D + 1])
            ao2v = ao2.rearrange("p (nt c) -> p nt c", c=P)
            for nt in range(NT):
                t = pv[nt // 4]
                tv = t[:, 0:4 * (D + 1)].rearrange("p (q s) -> p q s",
                                                   s=D + 1)
                nc.vector.tensor_scalar_mul(
                    ao2v[:, nt:nt + 1, hoff:hoff + D],
                    tv[:, nt % 4:nt % 4 + 1, 0:D],
                    rec[:, nt:nt + 1])

        def transpose_pair(pair, ao2):
            nc.sync.dma_start_transpose(
                out=aoT_sb[pair].rearrange("c (nt nn) -> c nt nn",
                                           nn=P)[:, :, :],
                in_=ao2[:])

        # ---- emission schedule ----
        # Work-queue driven: filler closures (next jt projection chunks,
        # v chains, ready PVs, final-proj chunks) are popped between QK
        # m-tile pairs so the PE never head-of-line blocks on the sps
        # double buffer while the exp stream paces the attention.
        pvps = None
        pending = {}   # h -> eps tiles
        ao2_t = None
        workq = []

        pv_done = set()

        def run_pv(hh):
            nonlocal ao2_t
            pv_done.add(hh)
            if hh % 2 == 0:
                ao2_t = small.tile([P, N], BF16, tag="ao2", name="ao2",
                                   bufs=2)
            head_pv(hh, pending.pop(hh), ao2_t)
            if hh % 2 == 1:
                transpose_pair(hh // 2, ao2_t)

        def enq_qkp(jt):
            qd = qkpool.tile([P, N], BF16, tag="qT", name="qT")
            kd = qkpool.tile([P, N], BF16, tag="kT", name="kT")
            workq.append((0.96, "p", lambda: qk_chunk(jt, qd, qwv, 0, True)))
            workq.append((0.96, "p", lambda: qk_chunk(jt, kd, kwv, 0, False)))
            workq.append((0.96, "p", lambda: qk_chunk(jt, qd, qwv, 1, True)))
            workq.append((0.96, "p", lambda: qk_chunk(jt, kd, kwv, 1, False)))
            return qd, kd

        def pop_filler(budget_us):
            while workq and budget_us > 0:
                cost, _, fn = workq.pop(0)
                fn()
                budget_us -= cost

        def final_a(jt, ch):
            csl = slice(ch * CHUNK, (ch + 1) * CHUNK)
            ps = pps.tile([P, CHUNK], F32, tag="pps", name="fps")
            for i, ct in enumerate(range(FSPLIT)):
                nc.tensor.matmul(
                    ps[:], lhsT=pw_t(ct, jt), rhs=aoT_sb[ct][:, csl],
                    start=(i == 0), stop=(ct == FSPLIT - 1))
            # alternate eviction engines: ACT idles once its exps drain
            if (jt + ch) % 2 == 0:
                nc.vector.tensor_scalar_add(
                    y_half[jt][:, csl], ps[:], pb_sb[:, jt:jt + 1])
            else:
                nc.scalar.activation(out=y_half[jt][:, csl], in_=ps[:],
                                     func=AFT.Identity,
                                     bias=pb_sb[:, jt:jt + 1])

        def final_b(jt):
            # wide chunk through the (now idle) score PSUM + one DMA; the
            # PSUM eviction + y_half combine alternates between a direct DVE
            # add and an ACT Identity eviction with a PE identity-matmul fold
            ps = sps.tile([P, N], F32, tag="sps", name="f2")
            fold = jt % 2 == 0   # ACT evicts these; ACT cannot add y_half
            for ch in range(NCH):
                csl = slice(ch * CHUNK, (ch + 1) * CHUNK)
                for i, ct in enumerate(range(FSPLIT, CT)):
                    nc.tensor.matmul(
                        ps[:, csl], lhsT=pw_t(ct, jt),
                        rhs=aoT_sb[ct][:, csl],
                        start=(i == 0), stop=(not fold and ch == NCH - 1),
                        skip_group_check=True)
                if fold:
                    # y_half folded in on the PE via identity accumulate so
                    # the eviction is a plain ACT Identity copy
                    nc.tensor.matmul(
                        ps[:, csl], lhsT=id_sb[:], rhs=y_half[jt][:, csl],
                        start=False, stop=(ch == NCH - 1),
                        skip_group_check=True)
            ob = fout.tile([P, N], BF16, tag="ob", name="ob", bufs=4)
            if not fold:
                nc.vector.tensor_add(ob[:], ps[:], y_half[jt][:])
            else:
                nc.scalar.activation(out=ob[:], in_=ps[:], func=AFT.Identity)
            nc.gpsimd.dma_start(out=yT[jt * P:(jt + 1) * P, :], in_=ob[:])

        # PV pull-forward: PV(k) at slot 6 + ceil(k/2) (FIFO, ring-safe for
        # LAG=6); PV(11) and the last final half drain after the loop.
        pv_slot = {6: [0], 7: [1, 2], 8: [3, 4], 9: [5, 6, 7], 10: [8, 9]}
        vq = [(mt, jc) for mt in range(NT) for jc in range(2)]
        vi = 0

        # head 0 with interleaved projection so the exp stream starts
        # as early as the DMAs allow
        qd0 = qkpool.tile([P, N], BF16, tag="qT", name="qT")
        kd0 = qkpool.tile([P, N], BF16, tag="kT", name="kT")
        qk_chunk(0, qd0, qwv, 0, True)
        qk_chunk(0, kd0, kwv, 0, False)
        qk_chunk(0, qd0, qwv, 1, True)
        pending[0] = head_qk(0, qd0, kd0, range(4))
        qk_chunk(0, kd0, kwv, 1, False)
        pending[0] += head_qk(0, qd0, kd0, range(4, NT))
        cur = (qd0, kd0)
        qk_next = None

        for h in range(H):
            jt = h // 2
            if h % 2 == 0 and h > 0:
                cur = qk_next
            q_t, k_t = cur

            # enqueue this slot's fillers
            if h % 2 == 1 and jt + 1 < CT:
                qk_next = enq_qkp(jt + 1)
            if h == 2:
                nc.gpsimd.dma_start(out=id_sb[:], in_=ident[:, :])
                # pw needed only by the output projection: late emission
                # keeps its transfer off the serialized DMA device while
                # the attention inputs stream in
                nc.gpsimd.dma_start(
                    out=pw_big.rearrange("p (ct c) -> p ct c", c=C),
                    in_=pwT.rearrange("(ct p) c -> p ct c", p=P))
                nc.gpsimd.dma_start(out=pb_sb[:], in_=pb[:, :])
            for _ in range(VPLAN.get(h, 0)):
                mt, jc = vq[vi]
                vi += 1
                workq.append((0.72, "v", lambda m=mt, j=jc: v_chain(m, j)))
            for k in pv_slot.get(h, []):
                workq.append((1.78, "pv", lambda kk=k: run_pv(kk)))
            # eps ring: head h's exps reuse head h-LAG's buffers; that PV
            # must be emitted first or the engines deadlock
            if h - LAG >= 0:
                while h - LAG not in pv_done:
                    cost, _, fn = workq.pop(0)
                    fn()

            if h == 10:
                for jj in range(2):
                    for cc in range(NCH):
                        workq.append((0.85, "f", lambda a=jj, b=cc: final_a(a, b)))

            if h == 11:
                # all queued PVs must precede PV(10)/PV(11) in the pvps
                # tag rotation or they deadlock behind norm(11)
                while any(k == "pv" for _, k, _ in workq):
                    cost, _, fn = workq.pop(0)
                    fn()
                # drain PV(10), then accumulate PV(11) incrementally behind
                # each pair of its own exps so only 2 m-tiles trail the
                # last exp
                run_pv(10)
                ao2_11 = ao2_t
                pv11 = pv_alloc()
                for mtp in range(4):
                    eps2 = head_qk(h, q_t, k_t, range(2 * mtp, 2 * mtp + 2))
                    if mtp == 3:
                        # fillers ahead of the exp-gated last PV pieces so
                        # the PE computes through the exp trail
                        pop_filler(2.2)
                    pv_mm(h, pv11, eps2, range(2 * mtp, 2 * mtp + 2),
                          mtp == 3)
                    if mtp < 3:
                        pop_filler(1.0)
            elif h > 0:
                for mtp in range(4):
                    pending.setdefault(h, [])
                    pending[h] += head_qk(h, q_t, k_t,
                                          range(2 * mtp, 2 * mtp + 2))
                    pop_filler(1.0)
            if h == 5:
                # all v chains must be emitted before the v pools close
                while any(k == "v" for _, k, _ in workq):
                    cost, _, fn = workq.pop(0)
                    fn()
                vstack.close()
                pvps = ctx.enter_context(
                    tc.tile_pool(name="pvps", bufs=1, space="PSUM"))

        pv_norm(11, pv11, ao2_11)
        transpose_pair(5, ao2_11)
        # remaining fillers + deferred stage-A cover the transpose latency
        pop_filler(1e9)
        for jt in range(CT):
            if jt + 2 < CT:
                for cc in range(NCH):
                    final_a(jt + 2, cc)
            final_b(jt)

    nc.compile()
    return nc


def _get_nc(use_f32r=True):
    key = ("nc", use_f32r)
    if key not in _CACHE:
        _CACHE[key] = build_nc(use_f32r)
    return _CACHE[key]


def _split8(a):
    """fp8 hi/lo split of float array a: a ~ hi + lo to ~fp8^2 precision."""
    e4 = ml_dtypes.float8_e4m3
    hi = a.astype(e4)
    lo = (a - hi.astype(np.float32)).astype(e4)
    return hi, lo


def kernel(x, qkv_w, qkv_b, q_w, q_b, q_A, q_B, v_w, v_b, v_A, v_B,
           proj_w, proj_b, _trace=False, _use_f32r=True):
    x = np.ascontiguousarray(np.asarray(x, dtype=np.float32))
    B = x.shape[0]
    assert x.shape == (8, N, C)

    qkv_w = np.asarray(qkv_w, np.float32)
    q_w = np.asarray(q_w, np.float32)
    q_b = np.asarray(q_b, np.float32)
    q_A = np.asarray(q_A, np.float32)
    q_B = np.asarray(q_B, np.float32)
    v_w = np.asarray(v_w, np.float32)
    v_b = np.asarray(v_b, np.float32)
    v_A = np.asarray(v_A, np.float32)
    v_B = np.asarray(v_B, np.float32)
    proj_w = np.asarray(proj_w, np.float32)
    proj_b = np.asarray(proj_b, np.float32)

    # exact algebraic folds (see module docstring)
    qw_eff = q_w + (q_B @ q_A) * (1.0 / R)
    vw_eff = v_w + (v_B @ v_A) * (1.0 / R)
    kw = qkv_w[C:2 * C]
    pb_eff = proj_b + proj_w @ v_b

    bf = ml_dtypes.bfloat16

    def pack_w(w, ncols):
        """W [o, c] -> [p, (cblk), hl, s, i, cols] fp8 pack, cblk=ncols/128.
        contraction row c = 256*s + 128*i + pp."""
        wt = (w.T * SW).astype(np.float32)            # [c, o]
        hi, lo = _split8(wt)
        out = np.empty((P, ncols // P, 2, 3, 2, P), ml_dtypes.float8_e4m3)
        for hl, part in ((0, hi), (1, lo)):
            m = part.reshape(3, 2, P, ncols)          # [s, i, pp, o]
            m = m.transpose(2, 3, 0, 1)               # [pp, o, s, i]
            out[:, :, hl] = m.reshape(P, ncols // P, P, 3, 2).transpose(
                0, 1, 3, 4, 2)
        return np.ascontiguousarray(out.reshape(P, -1))

    wq8 = pack_w(qw_eff, C)
    wk8 = pack_w(kw, C)

    # v pack keeps all 768 cols in one block: [p, hl, s, i, 768]
    vt = (vw_eff.T * SW).astype(np.float32)
    vhi, vlo = _split8(vt)
    wv8 = np.empty((P, 2, 3, 2, C), ml_dtypes.float8_e4m3)
    for hl, part in ((0, vhi), (1, vlo)):
        m = part.reshape(3, 2, P, C)
        wv8[:, hl] = m.transpose(2, 0, 1, 3)
    wv8 = np.ascontiguousarray(wv8.reshape(P, -1))

    common = {
        "wq8": wq8, "wk8": wk8, "wv8": wv8,
        "pwT": np.ascontiguousarray(proj_w.T.astype(bf)),
        "qb": np.ascontiguousarray(q_b.reshape(CT, P).T),
        "ident": np.eye(P, dtype=bf),
        "pb": np.ascontiguousarray(pb_eff.reshape(CT, P).T),
    }
    in_maps = []
    for i in range(B):
        xs = (x[i].T * SX).astype(np.float32)         # [c, n]
        xhi, xlo = _split8(xs)
        x8 = np.empty((P, 2, 3, 2, N), ml_dtypes.float8_e4m3)
        for hl, part in ((0, xhi), (1, xlo)):
            x8[:, hl] = part.reshape(3, 2, P, N).transpose(2, 0, 1, 3)
        in_maps.append(
            {"x8": np.ascontiguousarray(x8.reshape(P, -1)), **common})

    nc = _get_nc(_use_f32r)
    res = run_bass_kernel_spmd(nc, in_maps, list(range(B)), trace=_trace)

    out = np.empty((B, N, C), np.float32)
    for i in range(B):
        out[i] = np.asarray(res.results[i]["yT"], np.float32).T
    if _trace:
        return out, res
    return out
